# revision 69
# baseline (speedup 1.0000x reference)
"""AttentionPairBias Trainium2 kernel (8 NeuronCores, SPMD over query rows).

Sharding: the 768 query rows are split 96-per-core. Each core computes the
full output rows for its query slice; the host concatenates.

Device-side math (per core), exact LN algebra with centered weights:
  Wz'' = w*Wz - colsum(w*Wz)/CZ   (folds the LN mean term into the weights)
  pair_bias = rstd * (zT @ Wz'') + mask_bias   (+ colsum(b*Wz) in the exp)
  attention in "T-domain": scoresT[k, q] per head, softmax over the k
  (partition) axis; denominator via a ones-column in the v matmul; the
  pair-bias is accumulated into the scores PSUM by an identity matmul.

The z contraction keeps the tiny weight matrix STATIONARY on the PE (the
moving operand is z) so the tensor engine is matmul-bound rather than
LDWEIGHTS-bound; results land as [32-row groups, ij] stacked four deep in
one PSUM bank and are transposed back to key-partition layout with
full-width PE transposes.

Layout notes vs the earlier version of this kernel:
  - bias_sb is h-major [k, H, kt, q] so phase-C's bias-accumulate matmul
    reads a fully contiguous moving operand (the strided per-h gather was
    ~5x slower on the PE).
  - phase C processes 4 heads per PSUM tile (partition bands) so the
    softmax-denominator reciprocal + broadcast + gate/normalize muls run
    once per 4 heads on full-width tiles instead of per-head on 1..32
    partitions.
  - sigmoids are deferred to the start of phase C and sqrt/square/copy are
    the only activation functions used during phase B, so the scalar engine
    never reloads its activation table mid-stream.
  - constant/weight loads ride the gpsimd (SWDGE) ring, keeping the scalar
    and sync instruction queues free for compute / z-chunk DMAs.

The host passes z pre-transposed to [CZ=128, kt, q, kin] (bf16, key-tile
major so attention can start before the whole z pass finishes), plus
zero-padded / folded weight layouts.
"""

import os
import sys
import numpy as np

sys.path.insert(0, "/opt/trn_rl_repo")
os.environ.setdefault("MYCRO_LOCAL_CACHE", "1")

from ml_dtypes import bfloat16

# ---- problem constants (hardcoded per the harness contract) ----
B, N, C, CZ, H, CH = 1, 768, 384, 128, 16, 24
NCORES = 8
NQ = N // NCORES          # 96 query rows per core
CHP = 32                  # padded per-head width
HP = H * CHP              # 512 padded hc
EPS = 1e-5
INF = 1e9
KT = N // 128             # 6 key tiles
QG = 32                   # query rows per z-chunk
NQG = NQ // QG            # 3 query groups
NCHUNK = KT * NQG         # 18 chunks, key-tile major
NBLK = 4                  # PSUM row bands (32-row groups)
NB2 = 8                   # 512-wide moving blocks per chunk (2 col-halves)

_CACHE = {}


def _build_program(fold_mask=True):
    from contextlib import ExitStack
    import concourse.bass as bass
    import concourse.tile as tile
    from concourse import bacc, mybir

    f32 = mybir.dt.float32
    b16 = mybir.dt.bfloat16
    AF = mybir.ActivationFunctionType
    OP = mybir.AluOpType

    nc = bacc.Bacc("TRN2", target_bir_lowering=False, debug=False)

    # ---- DRAM I/O ----
    zt_d = nc.dram_tensor("zt", [CZ, KT * NQ * 128], b16, kind="ExternalInput")
    a_d = nc.dram_tensor("a_full", [N, C], b16, kind="ExternalInput")
    aq_d = nc.dram_tensor("a_q", [NQ, C], b16, kind="ExternalInput")
    wq_d = nc.dram_tensor("wq", [C, HP], b16, kind="ExternalInput")
    wk_d = nc.dram_tensor("wk", [C, HP], b16, kind="ExternalInput")
    wg_d = nc.dram_tensor("wg", [C, HP], b16, kind="ExternalInput")
    wv_d = nc.dram_tensor("wv", [C, C], b16, kind="ExternalInput")
    wo_d = nc.dram_tensor("wo", [HP, C], b16, kind="ExternalInput")
    bg_d = nc.dram_tensor("bg", [1, HP], b16, kind="ExternalInput")
    # host-folded z weights: col 0:16 = w*Wz - colsum(w*Wz)/CZ, col 16 = 1
    wza_d = nc.dram_tensor("wza", [CZ, 32], b16, kind="ExternalInput")
    # second stationary: col 17 = 1 (sum of squares), rest 0
    wzb_d = nc.dram_tensor("wzb", [CZ, 32], b16, kind="ExternalInput")
    tb_d = nc.dram_tensor("tbb", [128, H], f32, kind="ExternalInput")
    bqr_d = nc.dram_tensor("bqr", [1, HP], b16, kind="ExternalInput")
    bkr_d = nc.dram_tensor("bkr", [1, HP], b16, kind="ExternalInput")
    bvr_d = nc.dram_tensor("bvr", [1, C], b16, kind="ExternalInput")
    bo_d = nc.dram_tensor("bob", [128, C], f32, kind="ExternalInput")
    mask_d = nc.dram_tensor("maskt", [128, KT], f32, kind="ExternalInput")
    id_d = nc.dram_tensor("ident", [128, 128], b16, kind="ExternalInput")
    # band-broadcast matrix: bS[32j, 32j+c] = 1 (denominator row -> band)
    bS_d = nc.dram_tensor("bS", [128, 128], f32, kind="ExternalInput")
    out_d = nc.dram_tensor("out", [NQ, C], f32, kind="ExternalOutput")

    with tile.TileContext(nc) as tc, ExitStack() as ctx:
        const = ctx.enter_context(tc.tile_pool(name="const", bufs=1))

        # ------------- constant loads. The three tiles that gate the first
        # z-chunk and the transposes ride the scalar HWDGE ring (fast, ~2us
        # of descriptor-gen); everything else rides the gpsimd SWDGE ring so
        # the compute queues stay free. z + output ride the sync ring. -----
        wzaug = const.tile([CZ, 32], b16)
        nc.scalar.dma_start(wzaug, wza_d[:, :])
        wzsq = const.tile([CZ, 32], b16)
        nc.scalar.dma_start(wzsq, wzb_d[:, :])
        sb_id = const.tile([128, 128], b16)
        nc.scalar.dma_start(sb_id, id_d[:, :])

        a_sb = []
        for it in range(7):
            t = const.tile([128, C], b16, name=f"a{it}")
            if it < 6:
                nc.gpsimd.dma_start(t, a_d[128 * it:128 * (it + 1), :])
            else:
                nc.gpsimd.dma_start(t[0:NQ, :], aq_d[:, :])
            a_sb.append(t)

        wq_sb = []
        wk_sb = []
        wg_sb = []
        wv_sb = []
        for c in range(3):
            t = const.tile([128, HP], b16, name=f"wk{c}")
            nc.gpsimd.dma_start(t, wk_d[128 * c:128 * (c + 1), :])
            wk_sb.append(t)
            t = const.tile([128, C], b16, name=f"wv{c}")
            nc.gpsimd.dma_start(t, wv_d[128 * c:128 * (c + 1), :])
            wv_sb.append(t)
        for c in range(3):
            t = const.tile([128, HP], b16, name=f"wq{c}")
            nc.gpsimd.dma_start(t, wq_d[128 * c:128 * (c + 1), :])
            wq_sb.append(t)
            t = const.tile([128, HP], b16, name=f"wg{c}")
            nc.gpsimd.dma_start(t, wg_d[128 * c:128 * (c + 1), :])
            wg_sb.append(t)
        sb_bq = const.tile([1, HP], b16)
        nc.gpsimd.dma_start(sb_bq, bqr_d[:, :])
        sb_bk = const.tile([1, HP], b16)
        nc.gpsimd.dma_start(sb_bk, bkr_d[:, :])
        sb_bv = const.tile([1, C], b16)
        nc.gpsimd.dma_start(sb_bv, bvr_d[:, :])
        sb_bg = const.tile([1, HP], b16)
        nc.gpsimd.dma_start(sb_bg, bg_d[:, :])
        tb_b = const.tile([128, H], f32)
        nc.gpsimd.dma_start(tb_b, tb_d[:, :])
        # phase-C-only constants at the tail of the gpsimd queue
        wo_sb = []
        for c in range(4):
            t = const.tile([128, C], b16, name=f"wo{c}")
            nc.gpsimd.dma_start(t, wo_d[128 * c:128 * (c + 1), :])
            wo_sb.append(t)
        bo_b = const.tile([128, C], f32)
        nc.gpsimd.dma_start(bo_b, bo_d[:, :])
        sb_bS = const.tile([128, 128], f32)
        nc.gpsimd.dma_start(sb_bS, bS_d[:, :])
        if not fold_mask:
            sb_mask = const.tile([128, KT], f32)
            nc.gpsimd.dma_start(sb_mask, mask_d[:, :])

        # v_aug pad columns are 1.0 so the softmax-denominator reciprocal
        # stays finite on every partition row.
        v_aug = [const.tile([128, H, CHP], b16, name=f"vaug{t}") for t in range(KT)]
        for t in range(KT):
            nc.gpsimd.memset(v_aug[t], 1.0)

        # small derived constants
        ones_row_b96 = const.tile([1, NQ], b16)
        nc.vector.memset(ones_row_b96, 1.0)
        ones_row_b768 = const.tile([1, N], b16)
        nc.vector.memset(ones_row_b768, 1.0)
        eps_t = const.tile([128, 1], f32)
        nc.vector.memset(eps_t, EPS)
        if not fold_mask:
            # mask bias per key partition (folded into the stored pair-bias)
            mb = const.tile([128, KT], f32)
            nc.vector.tensor_scalar(mb, sb_mask, 1.0, INF, OP.subtract, OP.mult)

        # phase-B SBUF pools + PSUM pools open before phase A's (LIFO order:
        # A's pools release first, then B's, then phase C allocates)
        zpool = ctx.enter_context(tc.tile_pool(name="zpool", bufs=4))
        sqpool = ctx.enter_context(tc.tile_pool(name="sqpool", bufs=3))
        sbpool = ctx.enter_context(tc.tile_pool(name="sbp", bufs=3))
        zsm = ctx.enter_context(tc.tile_pool(name="zsmall", bufs=4))
        b_stack = ExitStack()
        psAp = b_stack.enter_context(tc.tile_pool(name="psA", bufs=2, space="PSUM"))
        psTp = b_stack.enter_context(tc.tile_pool(name="psT", bufs=2, space="PSUM"))

        # ------------- phase B chunk pipeline (emitted interleaved with
        # phase A below: engine queues are strict FIFO in emission order, so
        # early z-chunks must precede the projection matmuls in the tensor
        # queue or the PE idles for the whole phase-A lead-in) -------------
        # bias_sb layout: [k=128, kt, q, h] — h-inner matches the transpose
        # output, so the per-chunk store is a contiguous vector op; phase C
        # pays for the h-gather with a strided DVE read instead.
        bias_sb = const.tile([128, KT, NQ, H], b16)
        FW = QG * 128  # 2048 free elems per chunk
        state = {}

        def emit_chunk(chk):
            kt, qg = chk // NQG, chk % NQG
            zt_t = zpool.tile([128, FW], b16, tag="zt")
            nc.sync.dma_start(zt_t, zt_d[:, FW * chk:FW * (chk + 1)])
            # each chunk's square is column-split across V/S/G; early chunks
            # skip gpsimd (still draining DMA descriptor-gen) and lean on
            # scalar because the vector engine runs the LN(a) chains.
            sq_t = sqpool.tile([128, FW], b16, tag="sq")
            if chk < 6:
                vs, ss = 1536, FW
            else:
                vs, ss = 2048, 3584
            nc.vector.tensor_tensor(
                sq_t[:, 0:vs], zt_t[:, 0:vs], zt_t[:, 0:vs], OP.mult
            )
            nc.scalar.square(sq_t[:, vs:ss], zt_t[:, vs:ss])
            if ss < FW:
                nc.gpsimd.tensor_tensor(
                    sq_t[:, ss:FW], zt_t[:, ss:FW], zt_t[:, ss:FW], OP.mult
                )
            # contraction: weights stationary, z moving; results stacked in
            # 4 row bands x 2 column halves across a 2-bank PSUM tile
            psA = psAp.tile([128, 2, 512], f32, tag="psA")
            for b in range(NB2):
                band, half = b % NBLK, b // NBLK
                nc.tensor.matmul(
                    psA[32 * band:32 * band + 32, half, :], wzaug,
                    zt_t[:, 512 * b:512 * (b + 1)],
                    start=True, stop=False,
                    tile_position=(0, 32 * band), skip_group_check=True,
                )
            for b in range(NB2):
                band, half = b % NBLK, b // NBLK
                nc.tensor.matmul(
                    psA[32 * band:32 * band + 32, half, :], wzsq,
                    sq_t[:, 512 * b:512 * (b + 1)],
                    start=False, stop=True,
                    tile_position=(0, 32 * band), skip_group_check=True,
                )
            sbA = sbpool.tile([128, 2, 512], b16, tag="sbA")
            if chk % 3 == 0:
                nc.vector.tensor_copy(sbA, psA)
            else:
                nc.scalar.copy(sbA, psA)
            # transpose back to key-partition layout: psT[kin, (s, b, r)]
            # where slab s = 4*half + s1 covers sbA cols 128s..128(s+1)
            # (PE transposes; the DMA XBAR route measured ~100us slower)
            psT = psTp.tile([128, NB2, NBLK, 32], b16, tag="psT")
            sbA_f = sbA.rearrange("p a b -> p (a b)")
            for s in range(NB2):
                nc.tensor.transpose(
                    psT[:, s, :, :].rearrange("p a b -> p (a b)"),
                    sbA_f[:, 128 * s:128 * (s + 1)], sb_id,
                )
            # stats + bias on full-width batched views
            S = psT[:, :, :, H]                 # [128, s, b]
            Q = psT[:, :, :, H + 1]
            v1 = zsm.tile([128, NB2, NBLK], f32, tag="v1")
            nc.scalar.activation(v1, S, AF.Square, scale=1.0 / CZ)
            var = zsm.tile([128, NB2, NBLK], f32, tag="var")
            nc.vector.scalar_tensor_tensor(
                var, Q, 1.0 / CZ, v1, OP.mult, OP.subtract
            )
            stdv = zsm.tile([128, NB2, NBLK], f32, tag="stdv")
            nc.scalar.activation(stdv, var, AF.Sqrt, bias=eps_t)
            rstd = zsm.tile([128, NB2, NBLK], f32, tag="rstd")
            nc.vector.reciprocal(rstd, stdv)
            # pair q_local = 16*(s//4) + 4*band + (s%4): store per col-half
            # h2 (the (h2 s1) merge is not an affine AP dimension)
            for h2 in range(2):
                qo = qg * QG + 16 * h2
                outap = bias_sb[:, kt, qo:qo + 16, :].rearrange(
                    "p (b s1) h -> p s1 b h", s1=NBLK
                )
                slabs = slice(NBLK * h2, NBLK * (h2 + 1))
                rstd_b = rstd[:, slabs, :, None].broadcast_to(
                    [128, NBLK, NBLK, H]
                )
                if fold_mask:
                    # all-ones mask: bias write fused into the rstd scaling
                    nc.vector.tensor_tensor(
                        outap, psT[:, slabs, :, 0:H], rstd_b, OP.mult
                    )
                else:
                    tbig = zsm.tile([128, NBLK, NBLK, H], f32, tag="tbig")
                    nc.vector.tensor_tensor(
                        tbig, psT[:, slabs, :, 0:H], rstd_b, OP.mult
                    )
                    nc.vector.tensor_scalar(
                        outap, tbig, mb[:, kt:kt + 1], None, OP.add,
                    )
            state["last_rstd"] = rstd

        # first chunks ahead of everything: their tensor-queue slots only
        # need wza/wzsq (first on the scalar DMA ring), so the PE starts
        # within ~3us instead of waiting for the LN(a)+transpose chain
        for chk in range(0, 2):
            emit_chunk(chk)

        # ------------- phase A: LN(a) + projections, interleaved ----------
        a_stack = ExitStack()
        an_t = []
        apool = a_stack.enter_context(tc.tile_pool(name="apool", bufs=2))
        for it in range(7):
            p = 128 if it < 6 else NQ
            at = a_sb[it]
            stats = apool.tile([128, 6], f32, tag="stats")
            nc.vector.bn_stats(stats[0:p, :], at[0:p, :])
            mv = apool.tile([128, 2], f32, tag="mv")
            nc.vector.bn_aggr(mv[0:p, :], stats[0:p, :])
            stdv = apool.tile([128, 1], f32, tag="stdv")
            nc.scalar.activation(
                stdv[0:p, :], mv[0:p, 1:2], AF.Sqrt, bias=eps_t[0:p, :]
            )
            rstd = apool.tile([128, 1], f32, tag="rstd")
            nc.vector.reciprocal(rstd[0:p, :], stdv[0:p, :])
            ant = const.tile([128, C], b16, name=f"an{it}")
            nc.vector.tensor_scalar(
                ant[0:p, :], at[0:p, :], mv[0:p, 0:1], rstd[0:p, :],
                OP.subtract, OP.mult,
            )
            an_t.append(ant)

        anT = [const.tile([128, N], b16, name=f"anT{c}") for c in range(3)]
        anTq = [const.tile([128, NQ], b16, name=f"anTq{c}") for c in range(3)]
        pstr = a_stack.enter_context(tc.tile_pool(name="pstr", bufs=1, space="PSUM"))
        for it in range(6):
            for c in range(3):
                tp = pstr.tile([128, 128], b16, tag="tp")
                nc.tensor.transpose(tp, an_t[it][:, 128 * c:128 * (c + 1)], sb_id)
                nc.scalar.copy(anT[c][:, 128 * it:128 * (it + 1)], tp)
        for c in range(3):
            tp = pstr.tile([128, NQ], b16, tag="tp", name="tpq")
            nc.tensor.transpose(
                tp, an_t[6][0:NQ, 128 * c:128 * (c + 1)], sb_id[0:NQ, 0:NQ]
            )
            nc.scalar.copy(anTq[c], tp)

        for chk in range(2, 3):
            emit_chunk(chk)

        kTt = [const.tile([128, N], b16, name=f"kT{j}") for j in range(4)]
        qTt = [const.tile([128, NQ], b16, name=f"qT{j}") for j in range(4)]
        gpre = [const.tile([128, NQ], b16, name=f"gpre{j}") for j in range(4)]
        psp = a_stack.enter_context(tc.tile_pool(name="psproj", bufs=1, space="PSUM"))
        for j in range(4):
            for half in range(2):
                hw = 384
                kps = psp.tile([128, 384], f32, tag="pps", bufs=1, name=f"kps{j}_{half}")
                for c in range(3):
                    nc.tensor.matmul(
                        kps,
                        wk_sb[c][:, 128 * j:128 * (j + 1)],
                        anT[c][:, hw * half:hw * (half + 1)],
                        start=(c == 0), stop=False,
                    )
                nc.tensor.matmul(
                    kps, sb_bk[0:1, 128 * j:128 * (j + 1)],
                    ones_row_b768[0:1, hw * half:hw * (half + 1)],
                    start=False, stop=True,
                )
                nc.vector.tensor_copy(kTt[j][:, hw * half:hw * (half + 1)], kps)
        for t in range(KT):
            vps = psp.tile([128, C], f32, tag="pps", name="vps", bufs=1)
            for c in range(3):
                nc.tensor.matmul(
                    vps, anT[c][:, 128 * t:128 * (t + 1)], wv_sb[c],
                    start=(c == 0), stop=False,
                )
            nc.tensor.matmul(
                vps, ones_row_b768[0:1, 0:128], sb_bv,
                start=False, stop=True,
            )
            nc.vector.tensor_copy(
                v_aug[t][:, :, 1:CH + 1],
                vps.rearrange("p (h c) -> p h c", h=H),
            )

        for chk in range(3, 6):
            emit_chunk(chk)

        for j in range(4):
            qps = psp.tile([128, NQ], f32, tag="pps", name="qps", bufs=1)
            for c in range(3):
                nc.tensor.matmul(
                    qps, wq_sb[c][:, 128 * j:128 * (j + 1)], anTq[c],
                    start=(c == 0), stop=False,
                )
            nc.tensor.matmul(
                qps, sb_bq[0:1, 128 * j:128 * (j + 1)], ones_row_b96,
                start=False, stop=True,
            )
            nc.scalar.activation(qTt[j], qps, AF.Copy, scale=float(CH) ** -0.5)
            gps = psp.tile([128, NQ], f32, tag="pps", name="gps", bufs=1)
            for c in range(3):
                nc.tensor.matmul(
                    gps, wg_sb[c][:, 128 * j:128 * (j + 1)], anTq[c],
                    start=(c == 0), stop=False,
                )
            nc.tensor.matmul(
                gps, sb_bg[0:1, 128 * j:128 * (j + 1)], ones_row_b96,
                start=False, stop=True,
            )
            # gate applied as tanh in phase C (keeps the scalar act-table on
            # sqrt/square/copy throughout phase B)
            nc.vector.tensor_copy(gpre[j], gps)

        for chk in range(6, 9):
            emit_chunk(chk)
        a_stack.close()

        # ---- kg=0 scores + bias-add for heads 0-7 overlapped with chunks
        # 9-16 (one head per chunk: two head-blocks per chunk measured
        # bimodal, with in-order-queue head-of-line stalls). The exps stay
        # post-B (act-table conflict with the per-chunk sqrt); the biased
        # scores park in 8 small bf16 tiles. ------------------------------
        KG = 3   # key tiles per scores group
        NPRE = 9
        pp16 = [const.tile([128, KG, NQ], b16, name=f"pp{h}") for h in range(NPRE)]
        c_stack = ExitStack()
        scps0 = c_stack.enter_context(
            tc.tile_pool(name="scps0", bufs=2, space="PSUM")
        )

        def emit_head_kg0(h):
            cn, j = h // 4, h % 4
            jb = 32 * j
            sc = scps0.tile([128, KG, NQ], f32, tag="sc")
            for ks in range(KG):
                nc.tensor.matmul(
                    sc[:, ks, :],
                    kTt[cn][jb:jb + CHP, 128 * ks:128 * (ks + 1)],
                    qTt[cn][jb:jb + CHP, :],
                    start=(ks == 0), stop=(ks == KG - 1),
                    tile_position=(jb, 0), skip_group_check=True,
                )
            nc.vector.tensor_tensor(
                pp16[h], sc, bias_sb[:, 0:KG, :, h], OP.add,
            )

        for i, chk in enumerate(range(9, NCHUNK)):
            emit_chunk(chk)
            if i < NPRE:
                emit_head_kg0(i)
        last_rstd = state["last_rstd"]

        # ------------- phase C: attention -------------
        c_stack.close()
        b_stack.close()
        goT = [const.tile([128, NQ], b16, name=f"goT{c}") for c in range(4)]
        # gate as tanh: sigmoid(x) = (1 + tanh(x/2)) / 2; tanh shares the
        # scalar engine's activation table with exp, so phase C uses one
        # table. The 0.5 scale comes from an AP that data-depends on the
        # LAST z-chunk's stats, which pins these ops after phase B's
        # sqrt-table work (the scheduler would otherwise hoist them and
        # thrash the table).
        half_t = const.tile([128, 1], f32)
        nc.vector.tensor_scalar(
            half_t, last_rstd[:, 0, 0:1], 0.0, 0.5, OP.mult, OP.add
        )
        gTt = [const.tile([128, NQ], b16, name=f"gT{j}") for j in range(4)]
        for cn in range(4):
            nc.scalar.activation(gTt[cn], gpre[cn], AF.Tanh, scale=half_t)
        KG = 3   # key tiles per scores group
        with (
            tc.tile_pool(name="scps", bufs=3, space="PSUM") as scps,
            tc.tile_pool(name="otps", bufs=2, space="PSUM") as otps,
            tc.tile_pool(name="rbps", bufs=2, space="PSUM") as rbps,
            tc.tile_pool(name="pexp", bufs=4) as pexp,
            tc.tile_pool(name="rcpool", bufs=2) as rcpool,
            tc.tile_pool(name="tmppool", bufs=2) as tmppool,
            tc.tile_pool(name="psfin", bufs=1, space="PSUM") as psf,
        ):
            ops = psf.tile([NQ, C], f32)
            for cn in range(4):
                # 4 heads (bands jb=0/32/64/96) share one PSUM tile
                oT = otps.tile([128, NQ], f32, tag="oT")
                for j in range(4):
                    h = 4 * cn + j
                    jb = 32 * j
                    for kg in range(KT // KG):
                        if kg == 0 and h < NPRE:
                            # biased scores were prefolded during phase B
                            p_t = pexp.tile([128, KG, NQ], b16, tag="pt")
                            nc.scalar.activation(
                                p_t, pp16[h], AF.Exp, bias=tb_b[:, h:h + 1]
                            )
                        else:
                            sc = scps.tile([128, KG, NQ], f32, tag="sc")
                            for ks in range(KG):
                                kt = KG * kg + ks
                                nc.tensor.matmul(
                                    sc[:, ks, :],
                                    kTt[cn][jb:jb + CHP, 128 * kt:128 * (kt + 1)],
                                    qTt[cn][jb:jb + CHP, :],
                                    start=(ks == 0), stop=(ks == KG - 1),
                                    tile_position=(jb, 0), skip_group_check=True,
                                )
                            # pair-bias (+mask) added on the vector engine
                            # (cheaper than an identity matmul on the
                            # phase-C-bottleneck tensor engine)
                            p_pre = pexp.tile([128, KG, NQ], f32, tag="ppre")
                            nc.vector.tensor_tensor(
                                p_pre, sc,
                                bias_sb[:, KG * kg:KG * (kg + 1), :, h],
                                OP.add,
                            )
                            p_t = pexp.tile([128, KG, NQ], b16, tag="pt")
                            nc.scalar.activation(
                                p_t, p_pre, AF.Exp, bias=tb_b[:, h:h + 1]
                            )
                        for ks in range(KG):
                            kt = KG * kg + ks
                            nc.tensor.matmul(
                                oT[jb:jb + CHP, :], v_aug[kt][:, h, :],
                                p_t[:, ks, :],
                                start=(kt == 0), stop=(kt == KT - 1),
                                tile_position=(0, jb), skip_group_check=True,
                            )
                # batched softmax-denominator normalization for the 4 heads:
                # row 32j of oT is head 4cn+j's denominator (v_aug col 0).
                # max() guards the non-denominator rows against 1/0 = Inf,
                # which would poison the bS broadcast matmul (0 * Inf = NaN).
                # The *2 pairs with the (1 + tanh)/2 gate identity.
                rmax = rcpool.tile([128, NQ], f32, tag="rmax")
                nc.vector.tensor_scalar(rmax, oT, 1e-30, 2.0, OP.max, OP.mult)
                recip = rcpool.tile([128, NQ], f32, tag="recip")
                nc.vector.reciprocal(recip, rmax)
                rb = rbps.tile([128, NQ], f32, tag="rb")
                nc.tensor.matmul(
                    rb, sb_bS, recip, skip_group_check=True,
                )
                tmp = tmppool.tile([128, NQ], f32, tag="tmp")
                nc.vector.scalar_tensor_tensor(
                    tmp, gTt[cn], 1.0, oT, OP.add, OP.mult
                )
                nc.vector.tensor_tensor(goT[cn], tmp, rb, OP.mult)
                # output projection accumulates per group as it completes
                # instead of serializing in the kernel tail
                nc.tensor.matmul(
                    ops, goT[cn], wo_sb[cn], start=(cn == 0),
                    stop=(cn == 3), skip_group_check=True,
                )

            out_sb = const.tile([NQ, C], f32)
            nc.vector.tensor_tensor(out_sb, ops, bo_b[0:NQ, :], OP.add)
            nc.sync.dma_start(out_d[:, :], out_sb)

    nc.compile()
    return nc


def _get_program(fold_mask=True):
    key = ("nc", bool(fold_mask))
    if key not in _CACHE:
        _CACHE[key] = _build_program(fold_mask=fold_mask)
    return _CACHE[key]


def _pad_heads_cols(w, off):
    out = np.zeros((C, H, CHP), np.float32)
    out[:, :, off:off + CH] = np.asarray(w, np.float32).reshape(C, H, CH)
    return out.reshape(C, HP).astype(bfloat16)


def _host_inputs(inputs):
    a = np.asarray(inputs["a"], np.float32)
    z = np.asarray(inputs["z"], np.float32)
    mask = np.asarray(inputs["mask"], np.float32)
    Wz = np.asarray(inputs["Wz"], np.float32)
    Wo = np.asarray(inputs["Wo"], np.float32)
    bg = np.asarray(inputs["bg"], np.float32)
    lnzw = np.asarray(inputs["ln_z_w"], np.float32)
    lnzb = np.asarray(inputs["ln_z_b"], np.float32)
    lnaw = np.asarray(inputs["ln_a_w"], np.float32)
    lnab = np.asarray(inputs["ln_a_b"], np.float32)
    # fold LN(a)'s elementwise w into the projection weights; its b becomes
    # per-projection bias rows added via K=1 matmuls on-device
    Wq = lnaw[:, None] * np.asarray(inputs["Wq"], np.float32)
    Wk = lnaw[:, None] * np.asarray(inputs["Wk"], np.float32)
    Wg = lnaw[:, None] * np.asarray(inputs["Wg"], np.float32)
    Wv = lnaw[:, None] * np.asarray(inputs["Wv"], np.float32)
    bq = lnab @ np.asarray(inputs["Wq"], np.float32)
    bk = lnab @ np.asarray(inputs["Wk"], np.float32)
    bv = lnab @ np.asarray(inputs["Wv"], np.float32)
    bgf = bg + lnab @ np.asarray(inputs["Wg"], np.float32)

    wo_p = np.zeros((H, CHP, C), np.float32)
    wo_p[:, 1:CH + 1, :] = Wo.reshape(H, CH, C)
    bg_p = np.zeros((H, CHP), np.float32)
    bg_p[:, 1:CH + 1] = bgf.reshape(H, CH)
    def _pad_row(v, off):
        out = np.zeros((H, CHP), np.float32)
        out[:, off:off + CH] = v.reshape(H, CH)
        return out.reshape(1, HP).astype(bfloat16)

    # folded z-weight stationaries (bf16, consistency: center the bf16 values)
    wzp = (lnzw[:, None] * Wz).astype(bfloat16).astype(np.float32)
    wza = np.zeros((CZ, 32), np.float32)
    wza[:, 0:H] = wzp - wzp.sum(axis=0, keepdims=True) / CZ
    wza[:, H] = 1.0
    wzb = np.zeros((CZ, 32), np.float32)
    wzb[:, H + 1] = 1.0
    tb = (lnzb[:, None] * Wz).sum(axis=0)          # [H]

    bS = np.zeros((128, 128), np.float32)
    for j in range(4):
        bS[32 * j, 32 * j:32 * (j + 1)] = 1.0

    shared = {
        "a_full": np.ascontiguousarray(a[0]).astype(bfloat16),
        "wq": _pad_heads_cols(Wq, 0),
        "wk": _pad_heads_cols(Wk, 0),
        "wg": _pad_heads_cols(Wg, 1),
        "wv": Wv.astype(bfloat16),
        "wo": wo_p.reshape(HP, C).astype(bfloat16),
        "bg": bg_p.reshape(1, HP).astype(bfloat16),
        "bqr": _pad_row(bq, 0),
        "bkr": _pad_row(bk, 0),
        "bvr": bv.reshape(1, C).astype(bfloat16),
        "wza": wza.astype(bfloat16),
        "wzb": wzb.astype(bfloat16),
        "tbb": np.ascontiguousarray(np.broadcast_to(tb, (128, H))),
        "bob": np.ascontiguousarray(
            np.broadcast_to(np.asarray(inputs["bo"], np.float32), (128, C))),
        "maskt": np.ascontiguousarray(mask[0].reshape(KT, 128).T),
        "ident": np.eye(128, dtype=bfloat16),
        "bS": bS,
    }
    in_maps = []
    zb = z[0].astype(bfloat16)  # [N(q), N(k), CZ] bf16
    for core in range(NCORES):
        qs = slice(NQ * core, NQ * (core + 1))
        # [CZ, kt, q, kin] key-tile-major transposed layout
        zt = zb[qs].transpose(2, 1, 0).reshape(CZ, KT, 128, NQ)
        zt = np.ascontiguousarray(zt.transpose(0, 1, 3, 2)).reshape(CZ, -1)
        m = dict(shared)
        m["zt"] = zt
        m["a_q"] = np.ascontiguousarray(a[0, qs]).astype(bfloat16)
        in_maps.append(m)
    return in_maps


def _run(inputs, trace=False):
    from concourse.bass_utils import run_bass_kernel_spmd

    fold = bool(np.all(np.asarray(inputs["mask"], np.float32) == 1.0))
    nc = _get_program(fold_mask=fold)
    in_maps = _host_inputs(inputs)
    res = run_bass_kernel_spmd(
        nc, in_maps, core_ids=list(range(NCORES)), trace=trace
    )
    rows = [res.results[i]["out"] for i in range(NCORES)]
    out = np.concatenate(rows, axis=0).reshape(B, N, C).astype(np.float32)
    return out, res


def kernel(**inputs):
    out, _ = _run(inputs, trace=False)
    return out
